# revision 4
# baseline (speedup 1.0000x reference)
"""Paged-attention decode (vLLM-style) Bass kernel for Trainium2, 8 NeuronCores.

Sharding: KV heads across the 8 cores (tensor-parallel). Core h owns kv head h
and query heads 4h..4h+3 for ALL 32 sequences. Every core therefore runs an
IDENTICAL instruction stream (SPMD) — only its cache slice / q slice differ.

Per core:
  - host scatters the new k/v token into the caches, slices head h, and
    interleaves K|V per block into one [4096, 4096] f32 table
    (row = block: [K 16tok x 128d | V 16tok x 128d], 16 KiB).
  - device gathers up to 128 blocks per indirect DMA (one idx per partition,
    pad idx=65535 skipped via bounds_check), computes GQA attention:
      per 128-token chunk t:  K_T = PE-transpose(G_k[:, t]);
      sT[tok,4] = K_T.T @ qT   (PE, contraction over d)
      wT = exp(SCALE*sT + bias)   (ACT; bias = 0 / -1e30 validity mask)
      o[d,4]  += G_v[:, t].T @ wT  (PE, contraction over tokens)
      den[4,1] += wT.T @ ones      (PE)
    per sequence: out[4,128] = transpose(o) * (1/den)  -> DRAM.
"""

import os

import numpy as np

B, H, HKV, D = 32, 32, 8, 128
NUM_BLOCKS, BLOCK_SIZE, MAX_NUM_BLOCKS = 4096, 16, 256
SCALE = 0.08838834764831845
NCORES = 8
G = H // HKV  # 4 query heads per kv head
BPG = 128  # blocks per gather
TPC = 128  # tokens per compute chunk (= 128 blocks x 1 token-slot)
PAD_IDX = 65535
NEG = -1.0e30

LAST_EXEC_TIME_NS = None  # set when KERNEL_TRACE=1


def _plan(context_lens):
    """Static schedule shared by all cores: per-seq block counts and gathers."""
    nblocks = [int(-(-int(c) // BLOCK_SIZE)) if int(c) > 0 else 0 for c in context_lens]
    jobs = [b for b in range(B) if nblocks[b] > 0]
    ngathers = {b: -(-nblocks[b] // BPG) for b in jobs}
    return nblocks, jobs, ngathers


def _host_tables(block_tables, context_lens, nblocks, jobs, ngathers):
    ng_total = sum(ngathers[b] for b in jobs)
    idx = np.full((128, ng_total), PAD_IDX, dtype=np.int32)
    bias = np.full((128, ng_total * BLOCK_SIZE), NEG, dtype=np.float32)
    col = 0
    for b in jobs:
        nb = nblocks[b]
        ctx = int(context_lens[b])
        for g in range(ngathers[b]):
            lo = g * BPG
            n = min(BPG, nb - lo)
            idx[:n, col] = block_tables[b, lo : lo + n]
            # token at (block-slot lo+p, in-block offset t) is valid iff
            # (lo+p)*16 + t < ctx
            p = np.arange(128)
            for t in range(BLOCK_SIZE):
                valid = (lo + p) * BLOCK_SIZE + t < ctx
                bias[valid, col * BLOCK_SIZE + t] = 0.0
            col += 1
    return idx, bias, ng_total


def _build_program(nblocks, jobs, ngathers, ng_total):
    import concourse.mybir as mybir
    import concourse.tile as tile
    from concourse import bacc
    from concourse.bass import IndirectOffsetOnAxis

    f32 = mybir.dt.float32
    i32 = mybir.dt.int32
    Exp = mybir.ActivationFunctionType.Exp
    mult = mybir.AluOpType.mult

    nj = len(jobs)
    nc = bacc.Bacc("TRN2", target_bir_lowering=False)

    with tile.TileContext(nc) as tc:
        with tc.tile_pool(name="dram", bufs=1, space="DRAM") as dram:
            cache_t = dram.tile([NUM_BLOCKS, 2 * BLOCK_SIZE * D], f32,
                                kind="ExternalInput", name="cache", uniquify=False)
            idx_t = dram.tile([128, ng_total], i32,
                              kind="ExternalInput", name="idx", uniquify=False)
            bias_t = dram.tile([128, ng_total * BLOCK_SIZE], f32,
                               kind="ExternalInput", name="bias", uniquify=False)
            qT_t = dram.tile([D, B * G], f32,
                             kind="ExternalInput", name="qT", uniquify=False)
            ones_t = dram.tile([128, 1], f32,
                               kind="ExternalInput", name="ones", uniquify=False)
            ident_t = dram.tile([128, 128], f32,
                                kind="ExternalInput", name="ident", uniquify=False)
            o_t = dram.tile([nj, G, D], f32,
                            kind="ExternalOutput", name="o", uniquify=False)

        with (
            tc.tile_pool(name="resident", bufs=1) as rpool,
            tc.tile_pool(name="gpool", bufs=4) as gpool,
            tc.tile_pool(name="ktsb", bufs=3) as ktsb_pool,
            tc.tile_pool(name="wtsb", bufs=6) as wtsb_pool,
            tc.tile_pool(name="small", bufs=2) as small_pool,
            tc.tile_pool(name="ktps", bufs=2, space="PSUM") as ktps_pool,
            tc.tile_pool(name="stps", bufs=2, space="PSUM") as stps_pool,
            tc.tile_pool(name="ops", bufs=2, space="PSUM") as ops_pool,
            tc.tile_pool(name="denps", bufs=2, space="PSUM") as denps_pool,
        ):
            idx_sb = rpool.tile([128, ng_total], i32, tag="idx", name="idx_sb")
            bias_sb = rpool.tile([128, ng_total * BLOCK_SIZE], f32, tag="bias", name="bias_sb")
            qT_sb = rpool.tile([D, B * G], f32, tag="qT", name="qT_sb")
            ones_sb = rpool.tile([128, 1], f32, tag="ones", name="ones_sb")
            ident_sb = rpool.tile([128, 128], f32, tag="ident", name="ident_sb")
            nc.sync.dma_start(idx_sb[:], idx_t[:])
            nc.sync.dma_start(bias_sb[:], bias_t[:])
            nc.sync.dma_start(qT_sb[:], qT_t[:])
            nc.sync.dma_start(ones_sb[:], ones_t[:])
            nc.sync.dma_start(ident_sb[:], ident_t[:])

            col = 0
            for jb, b in enumerate(jobs):
                o_ps = ops_pool.tile([D, G], f32, tag="o")
                den_ps = denps_pool.tile([G, 1], f32, tag="den")
                nchunks = ngathers[b] * BLOCK_SIZE
                ci = 0
                for g in range(ngathers[b]):
                    gt = gpool.tile([128, 2 * BLOCK_SIZE * D], f32, tag="g")
                    nc.gpsimd.indirect_dma_start(
                        out=gt[:],
                        out_offset=None,
                        in_=cache_t[:],
                        in_offset=IndirectOffsetOnAxis(ap=idx_sb[:, col : col + 1], axis=0),
                        bounds_check=NUM_BLOCKS - 1,
                        oob_is_err=False,
                    )
                    for tp in range(BLOCK_SIZE // 2):
                        kt_ps = ktps_pool.tile([128, 256], f32, tag="kt")
                        kt_sb = ktsb_pool.tile([128, 256], f32, tag="ktsb")
                        for u in range(2):
                            t = 2 * tp + u
                            nc.tensor.transpose(
                                kt_ps[:, u * 128 : (u + 1) * 128],
                                gt[:, t * D : (t + 1) * D],
                                ident_sb[:],
                            )
                        nc.vector.tensor_copy(kt_sb[:], kt_ps[:])
                        for u in range(2):
                            t = 2 * tp + u
                            first = ci == 0
                            last = ci == nchunks - 1
                            st_ps = stps_pool.tile([128, G], f32, tag="st")
                            nc.tensor.matmul(
                                st_ps[:],
                                lhsT=kt_sb[:, u * 128 : (u + 1) * 128],
                                rhs=qT_sb[:, b * G : (b + 1) * G],
                                start=True,
                                stop=True,
                            )
                            wt_sb = wtsb_pool.tile([128, G], f32, tag="wt")
                            nc.scalar.activation(
                                wt_sb[:],
                                st_ps[:],
                                Exp,
                                bias=bias_sb[:, col * BLOCK_SIZE + t : col * BLOCK_SIZE + t + 1],
                                scale=SCALE,
                            )
                            nc.tensor.matmul(
                                o_ps[:],
                                lhsT=gt[:, (BLOCK_SIZE + t) * D : (BLOCK_SIZE + t + 1) * D],
                                rhs=wt_sb[:],
                                start=first,
                                stop=last,
                            )
                            nc.tensor.matmul(
                                den_ps[:],
                                lhsT=wt_sb[:],
                                rhs=ones_sb[:],
                                start=first,
                                stop=last,
                            )
                            ci += 1
                    col += 1
                # per-sequence epilogue
                o_sb = small_pool.tile([D, G], f32, tag="osb")
                nc.vector.tensor_copy(o_sb[:], o_ps[:])
                rec_sb = small_pool.tile([G, 1], f32, tag="rec")
                nc.vector.reciprocal(rec_sb[:], den_ps[:])
                ot_ps = stps_pool.tile([G, D], f32, tag="st")
                nc.tensor.transpose(ot_ps[:], o_sb[:], ident_sb[:])
                out_sb = small_pool.tile([G, D], f32, tag="out")
                nc.vector.tensor_scalar(
                    out_sb[:], ot_ps[:], rec_sb[:], None, op0=mult
                )
                nc.sync.dma_start(o_t[jb], out_sb[:])

    nc.compile()
    return nc


def kernel(q, k, v, k_cache, v_cache, slot_mapping, block_tables, context_lens):
    global LAST_EXEC_TIME_NS
    q = np.asarray(q, dtype=np.float32)
    k = np.asarray(k, dtype=np.float32)
    v = np.asarray(v, dtype=np.float32)
    k_cache = np.asarray(k_cache, dtype=np.float32)
    v_cache = np.asarray(v_cache, dtype=np.float32)
    slot_mapping = np.asarray(slot_mapping, dtype=np.int32)
    block_tables = np.asarray(block_tables, dtype=np.int32)
    context_lens = np.asarray(context_lens, dtype=np.int32)

    out = np.zeros((B, 1, H, D), dtype=np.float32)

    nblocks, jobs, ngathers = _plan(context_lens)
    if not jobs:
        return out

    # --- host prep ---
    kc = k_cache.reshape(-1, HKV, D).copy()
    vc = v_cache.reshape(-1, HKV, D).copy()
    kc[slot_mapping] = k
    vc[slot_mapping] = v
    kc = kc.reshape(NUM_BLOCKS, BLOCK_SIZE, HKV, D)
    vc = vc.reshape(NUM_BLOCKS, BLOCK_SIZE, HKV, D)

    idx, bias, ng_total = _host_tables(block_tables, context_lens, nblocks, jobs, ngathers)
    ones = np.ones((128, 1), dtype=np.float32)
    ident = np.eye(128, dtype=np.float32)

    in_maps = []
    for h in range(NCORES):
        cache_h = np.concatenate(
            [
                kc[:, :, h, :].reshape(NUM_BLOCKS, BLOCK_SIZE * D),
                vc[:, :, h, :].reshape(NUM_BLOCKS, BLOCK_SIZE * D),
            ],
            axis=1,
        )  # [4096, 4096]
        qT_h = np.ascontiguousarray(
            q[:, h * G : (h + 1) * G, :].transpose(2, 0, 1).reshape(D, B * G)
        )
        in_maps.append(
            {
                "cache": np.ascontiguousarray(cache_h),
                "idx": idx,
                "bias": bias,
                "qT": qT_h,
                "ones": ones,
                "ident": ident,
            }
        )

    nc = _build_program(nblocks, jobs, ngathers, ng_total)

    from concourse.bass_utils import run_bass_kernel_spmd

    res = run_bass_kernel_spmd(nc, in_maps, core_ids=list(range(NCORES)))
    LAST_EXEC_TIME_NS = res.exec_time_ns

    for h in range(NCORES):
        o_h = res.results[h]["o"]  # [nj, G, D]
        for jb, b in enumerate(jobs):
            out[b, 0, h * G : (h + 1) * G, :] = o_h[jb]
    return out
